# revision 12
# baseline (speedup 1.0000x reference)
"""CrystalDecoder Trainium2 kernel.

Strategy (8 NeuronCores, SPMD single program, per-core data):
- Graphs are split into 8 blocks of 32 (by graph id). Core k owns graph
  block k: its nodes (contiguous, since graph_id is sorted) and all edges
  whose src node belongs to the block (value-based edge binning, per the
  sharding hint). z + weights + graph_id are replicated.
- Node branch: feature-major MLP on PE; the per-node z_proj[graph_id]
  contribution is folded into the nd1 matmul via a clamp-matrix expand
  (graph_id is sorted, 32 local graphs) accumulated in PSUM.
- Edge branch: recon_edge depends only on (graph_id[src], graph_id[dst]).
  Edges are binned by that pair on the host (index bookkeeping only, per
  the sharding hint: both src and dst gathers become shard-local), each
  pair owning a fixed 64-edge slot. The device builds the 8192-pair
  table T3[gs_local, gd] from z and the MLP weights, replicates each
  pair's row across its slot's 64 edge positions, and writes every
  edge's 12-byte result; the host only inverse-permutes the
  device-written bytes.
"""

import numpy as np

# problem shapes (hardcoded; kernel.py must be self-contained)
B, N, E = 256, 200000, 1500000
LAT, HID, H2 = 32, 64, 128
NCORES, GPC = 8, 32
NCAP, NCHUNK = 26624, 52          # nodes/core capacity = 52*512
REPW = 64                         # edge slots per pair (max edges/pair)
NPAIR = 8192                      # 32 local graphs x 256

_CACHE = {}


def _build_program():
    import concourse.bass as bass
    import concourse.bacc as bacc
    import concourse.tile as tile
    from concourse import mybir

    f32 = mybir.dt.float32
    i32 = mybir.dt.int32
    i16 = mybir.dt.int16
    Alu = mybir.AluOpType
    Act = mybir.ActivationFunctionType

    nc = bacc.Bacc(
        "TRN2",
        target_bir_lowering=False,
        debug=False,
        enable_asserts=False,
        num_devices=NCORES,
    )

    def din(name, shape, dt=f32):
        return nc.dram_tensor(name, shape, dt, kind="ExternalInput")

    def dout(name, shape, dt=f32):
        return nc.dram_tensor(name, shape, dt, kind="ExternalOutput")

    # ---- inputs ----
    ne_T = din("ne_T", [H2, NCAP])            # node_emb shard, feature-major
    gidlf = din("gidlf", [NCAP])              # local graph id per node (f32)
    zT = din("zT", [LAT, B])
    z_slabT = din("z_slabT", [LAT, GPC])
    ident = din("ident", [128, 128])
    lp_w = din("lp_w", [LAT, H2])
    nep_w = din("nep_w", [H2, H2])
    nd1_w = din("nd1_w", [H2, HID])
    nd2_w = din("nd2_w", [HID, 4])
    ed1_ws = din("ed1_ws", [H2, HID])
    ed1_wd = din("ed1_wd", [H2, HID])
    ed2blk = din("ed2blk", [128, 6])
    en1_w = din("en1_w", [LAT, HID])
    en2_w = din("en2_w", [HID, 2])
    st1_w = din("st1_w", [LAT, HID])
    st2_w = din("st2_w", [HID, 9])
    lp_b = din("lp_b", [H2, 1])
    nep_b = din("nep_b", [H2, 1])
    nd1_b = din("nd1_b", [HID, 1])
    nd2_b = din("nd2_b", [4, 1])
    ed1_b = din("ed1_b", [HID, 1])
    ed2b2 = din("ed2b2", [6, 1])
    en1_b = din("en1_b", [HID, 1])
    en2_b = din("en2_b", [2, 1])
    st1_b = din("st1_b", [HID, 1])
    st2_b = din("st2_b", [9, 1])
    iota32m1 = din("iota32m1", [GPC, 1])      # g-1 for g in 0..31
    mdiffT = din("mdiffT", [GPC, GPC])        # (I - subdiag).T

    # ---- outputs ----
    rn_T = dout("rn_T", [4, NCAP])
    re_o = dout("re_o", [NPAIR * REPW, 3])
    en_T = dout("en_T", [2, B])
    st_T = dout("st_T", [9, B])

    # ---- scratch ----
    slab = nc.dram_tensor("slab", [GPC * B, 3], f32, kind="Internal")

    with tile.TileContext(nc) as tc:
        cst = tc.alloc_tile_pool(name="cst", bufs=1)

        def load_const(t, shape, dt=f32):
            tl = cst.tile(shape, dt, tag=t.name)
            nc.sync.dma_start(tl[:], t.ap()[:])
            return tl

        t_lp_w = load_const(lp_w, [LAT, H2])
        t_nep_w = load_const(nep_w, [H2, H2])
        t_nd1_w = load_const(nd1_w, [H2, HID])
        t_nd2_w = load_const(nd2_w, [HID, 4])
        t_ed1_ws = load_const(ed1_ws, [H2, HID])
        t_ed1_wd = load_const(ed1_wd, [H2, HID])
        t_ed2blk = load_const(ed2blk, [128, 6])
        t_en1_w = load_const(en1_w, [LAT, HID])
        t_en2_w = load_const(en2_w, [HID, 2])
        t_st1_w = load_const(st1_w, [LAT, HID])
        t_st2_w = load_const(st2_w, [HID, 9])
        t_zT = load_const(zT, [LAT, B])
        t_z_slabT = load_const(z_slabT, [LAT, GPC])
        t_id = load_const(ident, [128, 128])
        t_lp_b = load_const(lp_b, [H2, 1])
        t_nep_b = load_const(nep_b, [H2, 1])
        t_nd1_b = load_const(nd1_b, [HID, 1])
        t_nd2_b = load_const(nd2_b, [4, 1])
        t_ed1_b = load_const(ed1_b, [HID, 1])
        t_ed2b2 = load_const(ed2b2, [6, 1])
        t_en1_b = load_const(en1_b, [HID, 1])
        t_en2_b = load_const(en2_b, [2, 1])
        t_st1_b = load_const(st1_b, [HID, 1])
        t_st2_b = load_const(st2_b, [9, 1])
        t_i32m1 = load_const(iota32m1, [GPC, 1])
        t_mdT = load_const(mdiffT, [GPC, GPC])

        # ================= prologue: small tensors =================
        with tc.tile_pool(name="pro", bufs=1) as pro, \
             tc.tile_pool(name="prop", bufs=2, space="PSUM") as prop:
            # z_proj (full, transposed) and slab z_proj
            ps_zp = prop.tile([H2, B], f32, tag="pa")
            nc.tensor.matmul(ps_zp[:], lhsT=t_lp_w[:], rhs=t_zT[:],
                             start=True, stop=True)
            t_zpT = pro.tile([H2, B], f32)
            nc.scalar.activation(t_zpT[:], ps_zp[:], Act.Relu, bias=t_lp_b[:, 0:1])

            ps_zps = prop.tile([H2, GPC], f32, tag="pa")
            nc.tensor.matmul(ps_zps[:], lhsT=t_lp_w[:], rhs=t_z_slabT[:],
                             start=True, stop=True)
            t_zpsT = pro.tile([H2, GPC], f32)
            nc.scalar.activation(t_zpsT[:], ps_zps[:], Act.Relu, bias=t_lp_b[:, 0:1])

            # energy / stress heads
            for w1, b1, w2, b2, outd, m in (
                (t_en1_w, t_en1_b, t_en2_w, t_en2_b, en_T, 2),
                (t_st1_w, t_st1_b, t_st2_w, t_st2_b, st_T, 9),
            ):
                ps_h = prop.tile([HID, B], f32, tag="pa")
                nc.tensor.matmul(ps_h[:], lhsT=w1[:], rhs=t_zT[:],
                                 start=True, stop=True)
                t_h = pro.tile([HID, B], f32, tag="eh_s")
                nc.scalar.activation(t_h[:], ps_h[:], Act.Relu, bias=b1[:, 0:1])
                ps_o = prop.tile([m, B], f32, tag="pa")
                nc.tensor.matmul(ps_o[:], lhsT=w2[:], rhs=t_h[:],
                                 start=True, stop=True)
                t_o = pro.tile([m, B], f32, tag="eo_s")
                nc.vector.tensor_scalar(out=t_o[:], in0=ps_o[:],
                                        scalar1=b2[:, 0:1], scalar2=None,
                                        op0=Alu.add)
                nc.sync.dma_start(m_ap := outd.ap()[:], t_o[:])

            # C = zp_slab @ nd1_w  [32, 64]; D = row-diff of C
            ps_C = prop.tile([GPC, HID], f32, tag="pa")
            nc.tensor.matmul(ps_C[:], lhsT=t_zpsT[:], rhs=t_nd1_w[:],
                             start=True, stop=True)
            t_C = pro.tile([GPC, HID], f32)
            nc.vector.tensor_copy(out=t_C[:], in_=ps_C[:])
            ps_D = prop.tile([GPC, HID], f32, tag="pa")
            nc.tensor.matmul(ps_D[:], lhsT=t_mdT[:], rhs=t_C[:],
                             start=True, stop=True)
            # keep D in a const-pool tile (outlives prologue pool)
            t_Dk = cst.tile([GPC, HID], f32, tag="Dk")
            nc.vector.tensor_copy(out=t_Dk[:], in_=ps_D[:])

            # A2 = ed1_ws.T @ zp_slab + ed1_b  [64, 32]
            ps_A = prop.tile([HID, GPC], f32, tag="pa")
            nc.tensor.matmul(ps_A[:], lhsT=t_ed1_ws[:], rhs=t_zpsT[:],
                             start=True, stop=True)
            t_As2 = pro.tile([HID, GPC], f32)
            nc.vector.tensor_scalar(out=t_As2[:], in0=ps_A[:],
                                    scalar1=t_ed1_b[:, 0:1], scalar2=None,
                                    op0=Alu.add)
            # interleave: As2b[64a+f, j] = As2[f, 2j+a]
            t_As2b = pro.tile([128, 16], f32)
            a_v = t_As2[:].rearrange("f (j two) -> f two j", two=2)
            nc.vector.tensor_copy(out=t_As2b[0:64, :], in_=a_v[:, 0, :])
            nc.vector.tensor_copy(out=t_As2b[64:128, :], in_=a_v[:, 1, :])

            # B_T stacked twice  [128, 256]
            ps_B = prop.tile([HID, B], f32, tag="pa")
            nc.tensor.matmul(ps_B[:], lhsT=t_ed1_wd[:], rhs=t_zpT[:],
                             start=True, stop=True)
            t_B2 = pro.tile([128, B], f32)
            nc.vector.tensor_copy(out=t_B2[0:64, :], in_=ps_B[:])
            nc.vector.tensor_copy(out=t_B2[64:128, :], in_=ps_B[:])

            # pair table: 16 iterations cover 32 local graphs
            for j in range(16):
                t_pre = pro.tile([128, B], f32, tag="pre")
                nc.scalar.activation(t_pre[:], t_B2[:], Act.Relu,
                                     bias=t_As2b[:, j:j + 1])
                ps_6 = prop.tile([6, B], f32, tag="p6")
                nc.tensor.matmul(ps_6[:], lhsT=t_ed2blk[:], rhs=t_pre[:],
                                 start=True, stop=True)
                t_s6 = pro.tile([6, B], f32, tag="s6")
                nc.vector.tensor_scalar(out=t_s6[:], in0=ps_6[:],
                                        scalar1=t_ed2b2[:, 0:1], scalar2=None,
                                        op0=Alu.add)
                for half in range(2):
                    ps_t = prop.tile([128, 6], f32, tag="pt")
                    nc.tensor.transpose(ps_t[:], t_s6[:, half * 128:(half + 1) * 128],
                                        t_id[0:6, 0:6])
                    t_t = pro.tile([128, 6], f32, tag="tt")
                    nc.vector.tensor_copy(out=t_t[:], in_=ps_t[:])
                    base0 = (2 * j) * B + half * 128
                    base1 = (2 * j + 1) * B + half * 128
                    nc.sync.dma_start(slab.ap()[base0:base0 + 128, :],
                                      t_t[:, 0:3])
                    nc.sync.dma_start(slab.ap()[base1:base1 + 128, :],
                                      t_t[:, 3:6])


        # ============ edge pipeline: per-pair slot replication ============
        with tc.tile_pool(name="edg", bufs=2) as edg:
            t_slb = edg.tile([128, 64 * 3], f32, tag="slb")
            nc.sync.dma_start(
                t_slb[:].rearrange("p (s x) -> p s x", x=3),
                slab.ap()[:].rearrange("(p s) x -> p s x", p=128))
            sv = t_slb[:].rearrange("p (s x) -> p s x", x=3)
            CC = 16
            for cc in range(64 // CC):
                t_rep = edg.tile([128, CC * REPW * 3], f32, tag="rep")
                src = sv[:, cc * CC:(cc + 1) * CC, :].rearrange(
                    "p s (o x) -> p s o x", o=1).to_broadcast(
                    [128, CC, REPW, 3])
                nc.vector.tensor_copy(
                    out=t_rep[:].rearrange("p (s r x) -> p s r x",
                                           s=CC, r=REPW),
                    in_=src)
                dstv = re_o.ap()[:].rearrange(
                    "(p s r) x -> p s (r x)", p=128, s=64)
                nc.sync.dma_start(
                    dstv[:, cc * CC:(cc + 1) * CC, :],
                    t_rep[:].rearrange("p (s y) -> p s y", s=CC))

        # ================= node pipeline =================
        with tc.tile_pool(name="nod", bufs=3) as nod, \
             tc.tile_pool(name="nop", bufs=2, space="PSUM") as nop:
            for c in range(NCHUNK):
                n0 = c * 512
                t_ne = nod.tile([H2, 512], f32, tag="ne")
                nc.sync.dma_start(t_ne[:], ne_T.ap()[:, n0:n0 + 512])
                ps_n = nop.tile([H2, 512], f32, tag="pn")
                nc.tensor.matmul(ps_n[:], lhsT=t_nep_w[:], rhs=t_ne[:],
                                 start=True, stop=True)
                t_n2 = nod.tile([H2, 512], f32, tag="n2")
                nc.scalar.activation(t_n2[:], ps_n[:], Act.Relu,
                                     bias=t_nep_b[:, 0:1])
                ps_1 = nop.tile([HID, 512], f32, tag="p1")
                nc.tensor.matmul(ps_1[:], lhsT=t_nd1_w[:], rhs=t_n2[:],
                                 start=True, stop=False)
                t_gb = nod.tile([GPC, 512], f32, tag="gb")
                nc.sync.dma_start(t_gb[:], gidlf.ap()[n0:n0 + 512].rearrange(
                    "(o f) -> o f", o=1).partition_broadcast(GPC))
                t_cm = nod.tile([GPC, 512], f32, tag="cm")
                nc.vector.tensor_scalar(out=t_cm[:], in0=t_gb[:],
                                        scalar1=t_i32m1[:, 0:1], scalar2=0.0,
                                        op0=Alu.subtract, op1=Alu.max)
                nc.vector.tensor_scalar(out=t_cm[:], in0=t_cm[:], scalar1=1.0,
                                        scalar2=None, op0=Alu.min)
                nc.tensor.matmul(ps_1[:], lhsT=t_Dk[:], rhs=t_cm[:],
                                 start=False, stop=True)
                t_t1 = nod.tile([HID, 512], f32, tag="t1")
                nc.scalar.activation(t_t1[:], ps_1[:], Act.Relu,
                                     bias=t_nd1_b[:, 0:1])
                ps_4 = nop.tile([4, 512], f32, tag="p4")
                nc.tensor.matmul(ps_4[:], lhsT=t_nd2_w[:], rhs=t_t1[:],
                                 start=True, stop=True)
                t_o4 = nod.tile([4, 512], f32, tag="o4")
                nc.vector.tensor_scalar(out=t_o4[:], in0=ps_4[:],
                                        scalar1=t_nd2_b[:, 0:1], scalar2=None,
                                        op0=Alu.add)
                nc.sync.dma_start(rn_T.ap()[:, n0:n0 + 512], t_o4[:])

        cst.release()

    nc.compile()
    return nc


def _host_prep(inputs):
    """Shard/pad/bin inputs per core. Returns (in_maps, meta)."""
    gid = np.asarray(inputs["graph_id"]).astype(np.int64)
    src = np.asarray(inputs["src"]).astype(np.int64)
    dst = np.asarray(inputs["dst"]).astype(np.int64)
    z = np.asarray(inputs["z"], np.float32)
    ne = np.asarray(inputs["node_emb"], np.float32)

    starts = np.searchsorted(gid, np.arange(257)).astype(np.int64)  # [257]
    gs = (np.searchsorted(starts, src, side="right") - 1).astype(np.int64)
    gd = (np.searchsorted(starts, dst, side="right") - 1).astype(np.int64)
    core_of = gs >> 5

    ident = np.eye(128, dtype=np.float32)

    ed1_w = np.asarray(inputs["ed1_w"], np.float32)
    ed2_w = np.asarray(inputs["ed2_w"], np.float32)
    ed2blk = np.zeros((128, 6), np.float32)
    ed2blk[0:64, 0:3] = ed2_w
    ed2blk[64:128, 3:6] = ed2_w

    def col(name):
        return np.asarray(inputs[name], np.float32).reshape(-1, 1)

    shared = dict(
        ident=ident,
        zT=np.ascontiguousarray(z.T),
        lp_w=np.asarray(inputs["lp_w"], np.float32),
        nep_w=np.asarray(inputs["nep_w"], np.float32),
        nd1_w=np.asarray(inputs["nd1_w"], np.float32),
        nd2_w=np.asarray(inputs["nd2_w"], np.float32),
        ed1_ws=np.ascontiguousarray(ed1_w[:H2]),
        ed1_wd=np.ascontiguousarray(ed1_w[H2:]),
        ed2blk=ed2blk,
        en1_w=np.asarray(inputs["en1_w"], np.float32),
        en2_w=np.asarray(inputs["en2_w"], np.float32),
        st1_w=np.asarray(inputs["st1_w"], np.float32),
        st2_w=np.asarray(inputs["st2_w"], np.float32),
        lp_b=col("lp_b"), nep_b=col("nep_b"), nd1_b=col("nd1_b"),
        nd2_b=col("nd2_b"), ed1_b=col("ed1_b"),
        ed2b2=np.concatenate([inputs["ed2_b"], inputs["ed2_b"]]).astype(
            np.float32).reshape(6, 1),
        en1_b=col("en1_b"), en2_b=col("en2_b"),
        st1_b=col("st1_b"), st2_b=col("st2_b"),
        iota32m1=(np.arange(GPC, dtype=np.float32) - 1.0).reshape(GPC, 1),
        mdiffT=np.ascontiguousarray(
            (np.eye(GPC, dtype=np.float32)
             - np.eye(GPC, k=-1, dtype=np.float32)).T),
    )

    in_maps, meta = [], []
    for k in range(NCORES):
        nb_k = int(starts[32 * k])
        cnt = int(starts[32 * (k + 1)] - starts[32 * k])
        assert cnt <= NCAP, f"node capacity exceeded: {cnt}"
        ne_T = np.zeros((H2, NCAP), np.float32)
        ne_T[:, :cnt] = ne[nb_k:nb_k + cnt].T
        gidlf = np.full(NCAP, float(GPC - 1), np.float32)
        gidlf[:cnt] = (gid[nb_k:nb_k + cnt] - 32 * k).astype(np.float32)

        e_idx = np.nonzero(core_of == k)[0]
        pl = (gs[e_idx] - 32 * k) * 256 + gd[e_idx]      # local pair id
        order = np.argsort(pl, kind="stable")
        pls = pl[order]
        counts = np.bincount(pls, minlength=NPAIR)
        assert counts.max() <= REPW, f"pair overflow: {counts.max()}"
        grp_start = np.concatenate([[0], np.cumsum(counts)[:-1]])
        rank = np.arange(len(pls)) - grp_start[pls]
        padded_pos = pls * REPW + rank

        m = dict(shared)
        m.update(ne_T=ne_T, gidlf=gidlf,
                 z_slabT=np.ascontiguousarray(z[32 * k:32 * (k + 1)].T))
        in_maps.append(m)
        meta.append((nb_k, cnt, e_idx[order], padded_pos))
    return in_maps, meta


def kernel(**inputs):
    import os
    from concourse.bass_utils import run_bass_kernel_spmd

    if "nc" not in _CACHE:
        _CACHE["nc"] = _build_program()
    nc = _CACHE["nc"]

    in_maps, meta = _host_prep(inputs)
    res = run_bass_kernel_spmd(
        nc, in_maps, core_ids=list(range(NCORES)),
        trace=bool(os.environ.get("KBENCH_TRACE")))
    _CACHE["last_res"] = res
    outs = res.results

    recon_node = np.zeros((N, 4), np.float32)
    recon_edge = np.zeros((E, 3), np.float32)
    for k in range(NCORES):
        nb_k, cnt, e_idx, padded_pos = meta[k]
        recon_node[nb_k:nb_k + cnt] = outs[k]["rn_T"][:, :cnt].T
        recon_edge[e_idx] = outs[k]["re_o"][padded_pos]
    pred_energy = np.ascontiguousarray(outs[0]["en_T"].T)
    pred_stress = np.ascontiguousarray(outs[0]["st_T"].T)
    return recon_node, recon_edge, pred_energy, pred_stress
